# revision 14
# baseline (speedup 1.0000x reference)
# Trainium2 Bass kernel for nn_BayesianExpectationTransformerLayer.
#
# Math: attention with no positional encoding / masking is permutation-
# equivariant: _attention(x[:, perm, :]) == _attention(x)[:, perm, :].
# Hence each permuted pass, after applying the inverse permutation, equals
# the standard attention output exactly, and the whole module collapses to
#     out = c * (attention(x) @ Wo^T + bo),
#     c   = (1 - w) + w * variance_reduction_weight,
#     w   = clip(length_adaptive_weight * log(S)/S, 0.01, 1.0)
# We verify on the host that `perms` really are permutations of [0, S);
# if they are not (general fallback), we run the same device kernel once
# per pass (standard + K permuted copies) and combine on the host.
#
# Device strategy (8 NeuronCores, SPMD, tensor-parallel over heads, NO
# collective):
#   - core c owns heads 2c, 2c+1 (feature slice F = 128 of D = 1024).
#     Instead of an AllToAll re-shard for the out-projection, each core
#     computes the FULL [R, D] partial out-projection restricted to its
#     own 128 features (contraction F = 128 -> identical PE cost to a
#     1/8-rows x 1024-contraction projection) and the host sums the 8
#     bf16 partials.  No collective, no re-shard DMAs, no drain tail.
#   - per phase (= one batch): Q^T/K^T/V^T = [F, S] projections (bf16),
#     V into natural layout via PE transposes with an appended ones
#     column (softmax denominator), S^T = K Q^T scores per head, exp to
#     bf16 (no max-subtraction: |scores| < 7 for this data), AV with V
#     stationary accumulating feature-major with the denominator in row
#     HD, 1/denom applied on DVE, out-projection with at_sb token-chunks
#     stationary against the core's 128 x 1024 Wo row-slice.
#   - scheduling: the 16 st->exp score steps of phase i are ACT-bound
#     (~16 x 1.04us of exp).  All other PE work -- next phase's
#     projections+transposes, previous phase's second-head AV and
#     out-projection, this phase's first-head AV -- is emitted as
#     single-matmul "fillers" between score steps, so the PE queue never
#     drains while ACT works.  PSUM zero-region semantics allow unrelated
#     matmuls (other banks) to interleave inside accumulation chains.
#   - PSUM budget/partition: st ring 2x4KB + pp ring 2x2KB (proj ps,
#     V-transpose targets via bitcast, out-proj po) + av ring 2x2KB
#     (at_raw) = 16KB exactly.
#   - out-proj eviction fp32->bf16 on GPSIMD (Pool is otherwise idle),
#     output DMA on the DVE queue; x chunks on SP; weights on SP/ACT.

import os
import sys

for _p in ("/opt/trn_rl_repo", "/root/.axon_site/_ro/trn_rl_repo"):
    if os.path.isdir(_p) and _p not in sys.path:
        sys.path.append(_p)

from collections import deque

import numpy as np

import concourse.bass as bass
import concourse.mybir as mybir
import concourse.tile as tile
from concourse import bacc
from concourse.bass import ts
from concourse.bass_utils import run_bass_kernel_spmd
from concourse.masks import make_identity

B, S, D = 2, 1024, 1024
H, HD = 16, 64
KPERM = 20
NCORES = 8
HPC = H // NCORES          # heads per core = 2
F = HPC * HD               # per-core feature slice = 128
R = B * S                  # 2048 rows
NKC = S // 128             # 8 k-chunks per sequence
NQC2 = S // 512            # 2 q-chunks of 512
FP32 = mybir.dt.float32
BF16 = mybir.dt.bfloat16

TRACE = False              # set True to attempt an NTFF profile capture
LAST = None                # BassKernelResults of the last run

_CACHED = {}


def _build(reps=1):
    """Build the SPMD Bass program (identical on all 8 cores).

    reps > 1 repeats the whole computation serially in one NEFF (used
    only for timing: per-rep slope isolates device time from dispatch).
    """
    nc = bacc.Bacc(None)

    xT = nc.declare_dram_parameter("xT", [D, R], BF16, isOutput=False)
    # host pre-rearranges W*T [D, F] -> [128, 8, F] so each partition's
    # row is contiguous: 128 x 2KB descriptors instead of 1024 x 256B.
    wq3 = nc.declare_dram_parameter("wq3", [128, 8, F], BF16, isOutput=False)
    wk3 = nc.declare_dram_parameter("wk3", [128, 8, F], BF16, isOutput=False)
    wv3 = nc.declare_dram_parameter("wv3", [128, 8, F], BF16, isOutput=False)
    woS = nc.declare_dram_parameter("woS", [F, D], BF16, isOutput=False)
    bqs = nc.declare_dram_parameter("bqs", [F, 1], FP32, isOutput=False)
    bks = nc.declare_dram_parameter("bks", [F, 1], FP32, isOutput=False)
    bvb = nc.declare_dram_parameter("bvb", [128, HPC, HD], FP32, isOutput=False)
    out = nc.declare_dram_parameter("out", [R, D], BF16, isOutput=True)

    Exp = mybir.ActivationFunctionType.Exp

    with tile.TileContext(nc) as tc:
        with (
            # PSUM pools first so their arena layout is bank-aligned:
            # st 2x4KB | pp 2x2KB | av 2x2KB per partition.
            tc.tile_pool(name="ps_st", bufs=2, space="PSUM") as ps_st,
            tc.tile_pool(name="ps_pp", bufs=2, space="PSUM") as ps_pp,
            tc.tile_pool(name="ps_av", bufs=2, space="PSUM") as ps_av,
            tc.tile_pool(name="const", bufs=1) as cpool,
            tc.tile_pool(name="xt", bufs=10) as xtpool,
            tc.tile_pool(name="qkv", bufs=2) as qkvpool,
            tc.tile_pool(name="vnat", bufs=2) as vpool,
            tc.tile_pool(name="pt", bufs=2) as ptpool,
            tc.tile_pool(name="at", bufs=2) as atpool,
            tc.tile_pool(name="sm", bufs=4) as smpool,
            tc.tile_pool(name="bc", bufs=3) as bcpool,
            tc.tile_pool(name="osb", bufs=6) as opool,
        ):
            # ---- constants ----
            ident = cpool.tile([128, 128], FP32, tag="ident")
            make_identity(nc, ident[:])
            ident_bf = cpool.tile([128, 128], BF16, tag="ident_bf")
            nc.gpsimd.tensor_copy(ident_bf[:], ident[:])

            wq_sb = cpool.tile([128, 8, F], BF16, tag="wq")
            wk_sb = cpool.tile([128, 8, F], BF16, tag="wk")
            wv_sb = cpool.tile([128, 8, F], BF16, tag="wv")
            wo_sb = cpool.tile([128, D], BF16, tag="wo")
            # wq on SP so it lands before the first xt chunks; wk/wv/wo on
            # the ACT HWDGE queue (idle until the first exp, ~12us in) so
            # the SP queue reaches phase 0's x chunks immediately.
            nc.sync.dma_start(wq_sb[:], wq3[:])
            nc.scalar.dma_start(wk_sb[:], wk3[:])
            nc.scalar.dma_start(wv_sb[:], wv3[:])
            nc.scalar.dma_start(wo_sb[:], woS[:])
            bq_sb = cpool.tile([F, 1], FP32, tag="bq")
            bk_sb = cpool.tile([F, 1], FP32, tag="bk")
            bv_sb = cpool.tile([128, HPC, HD], FP32, tag="bv")
            nc.gpsimd.dma_start(bq_sb[:], bqs[:])
            nc.gpsimd.dma_start(bk_sb[:], bks[:])
            nc.gpsimd.dma_start(bv_sb[:], bvb[:])

            xTr = xT[:].rearrange("(c p) r -> p c r", p=128)

            state = {}

            def proj_closures(i, b):
                """Fillers that compute phase i's Q/K/V projections, V
                transposes and V-natural assembly.  One matmul each."""
                QT = qkvpool.tile([128, S], BF16, tag="QT", name=f"QT{i}")
                KT = qkvpool.tile([128, S], BF16, tag="KT", name=f"KT{i}")
                VT = qkvpool.tile([128, S], BF16, tag="VT", name=f"VT{i}")
                V0 = vpool.tile([128, NKC, HD + 1], BF16, tag="V0", name=f"V0{i}")
                V1 = vpool.tile([128, NKC, HD + 1], BF16, tag="V1", name=f"V1{i}")
                state[i] = {"QT": QT, "KT": KT, "VT": VT, "V0": V0, "V1": V1}
                cls = []

                def start_loads():
                    # [128, 2, 512] tiles: one DMA covers two d-chunks --
                    # fewer DMA issues so arrivals keep pace with the PE's
                    # 427ns-per-chunk consumption during the prologue.
                    xts = []
                    for half, rc in enumerate((2 * b, 2 * b + 1)):
                        row = []
                        for dp in range(4):
                            xt_t = xtpool.tile(
                                [128, 2, 512], BF16, tag="xt",
                                name=f"xt{i}_{half}_{dp}",
                            )
                            nc.sync.dma_start(
                                xt_t[:], xTr[:, 2 * dp : 2 * dp + 2, ts(rc, 512)]
                            )
                            row.append(xt_t)
                        xts.append(row)
                    nc.gpsimd.memset(V0[:, :, HD : HD + 1], 1.0)
                    nc.gpsimd.memset(V1[:, :, HD : HD + 1], 1.0)
                    return xts

                xts_box = {}

                def first():
                    xts_box["xts"] = start_loads()
                cls.append(first)

                holder = {}

                def mk_mm(half, w_sb, dc, newtile):
                    def go():
                        if newtile:
                            holder["ps"] = ps_pp.tile(
                                [128, 512], FP32, tag="pp",
                                name=f"proj{i}_{half}_{id(w_sb)}",
                            )
                        nc.tensor.matmul(
                            holder["ps"][:],
                            lhsT=w_sb[:, dc, :],
                            rhs=xts_box["xts"][half][dc // 2][:, dc % 2, :],
                            start=(dc == 0), stop=(dc == 7),
                        )
                    return go

                def mk_evict(half, b_sb, dst):
                    def go():
                        if b_sb is not None:
                            nc.vector.tensor_scalar_add(
                                dst[:, ts(half, 512)], holder["ps"][:],
                                b_sb[:, 0:1],
                            )
                        else:
                            nc.vector.tensor_copy(
                                dst[:, ts(half, 512)], holder["ps"][:]
                            )
                    return go

                for half in range(2):
                    for w_sb, b_sb, dst in (
                        (wq_sb, bq_sb, QT), (wk_sb, bk_sb, KT),
                        (wv_sb, None, VT),
                    ):
                        for dc in range(8):
                            cls.append(mk_mm(half, w_sb, dc, dc == 0))
                        cls.append(mk_evict(half, b_sb, dst))

                tp_box = {}

                def mk_tp(kc):
                    def go():
                        tpf = ps_pp.tile(
                            [128, 512], FP32, tag="pp", name=f"tp{i}_{kc}"
                        )
                        tpb = tpf[:].bitcast(BF16)
                        nc.tensor.transpose(
                            tpb[:, 0:128], VT[:, ts(kc, 128)], ident_bf[:]
                        )
                        tp_box[kc] = tpb
                    return go

                def mk_vadd(kc):
                    def go():
                        tpb = tp_box.pop(kc)
                        for h, Vh in ((0, V0), (1, V1)):
                            nc.vector.tensor_add(
                                Vh[:, kc, 0:HD], tpb[:, ts(h, HD)],
                                bv_sb[:, h, :],
                            )
                    return go

                for kc in range(NKC):
                    cls.append(mk_tp(kc))
                    cls.append(mk_vadd(kc))
                return cls

            def av_closures(i, h):
                """Fillers for phase i's AV of head h: 2 (qc2) groups of 8
                accumulating matmuls; the 8th of each group emits the
                1/denom normalize into at_sb."""
                ph = state[i]
                Vh = ph["V0"] if h == 0 else ph["V1"]
                if "at_sb" not in ph:
                    ph["at_sb"] = atpool.tile(
                        [128, S], BF16, tag="at", name=f"at{i}"
                    )
                at_sb = ph["at_sb"]
                ptb = ph["ptb"]
                cls = []
                holder = {}

                def mk_mm(qc2, kc):
                    def go():
                        if kc == 0:
                            holder["ar"] = ps_av.tile(
                                [128, 512], FP32, tag="av",
                                name=f"ar{i}_{h}_{qc2}",
                            )
                        nc.tensor.matmul(
                            holder["ar"][0 : HD + 1, :],
                            lhsT=Vh[:, kc, :],
                            rhs=ptb[:, h, kc, ts(qc2, 512)],
                            start=(kc == 0), stop=(kc == 7),
                        )
                    return go

                def mk_norm(qc2):
                    def go():
                        ar = holder["ar"]
                        rrow = smpool.tile(
                            [1, 512], FP32, tag="rrow", name=f"rr{i}_{h}_{qc2}"
                        )
                        nc.vector.reciprocal(rrow[:], ar[HD : HD + 1, :])
                        bcast = bcpool.tile(
                            [HD, 512], FP32, tag="bcast", name=f"bc{i}_{h}_{qc2}"
                        )
                        nc.gpsimd.partition_broadcast(bcast[:], rrow[:])
                        nc.vector.tensor_mul(
                            at_sb[ts(h, HD), ts(qc2, 512)], ar[0:HD, :],
                            bcast[:],
                        )
                    return go

                for qc2 in range(NQC2):
                    for kc in range(NKC):
                        cls.append(mk_mm(qc2, kc))
                    cls.append(mk_norm(qc2))
                return cls

            def outproj_closures(i, b):
                """Fillers for phase i's partial out-projection: per token
                chunk tc and 512-wide d-slice, one matmul + Pool eviction
                + output DMA."""
                at_sb = state[i]["at_sb"]
                cls = []

                def mk(tc, dc2):
                    def go():
                        po = ps_pp.tile(
                            [128, 512], FP32, tag="pp", name=f"po{i}_{tc}_{dc2}"
                        )
                        nc.tensor.matmul(
                            po[:],
                            lhsT=at_sb[:, ts(tc, 128)],
                            rhs=wo_sb[:, ts(dc2, 512)],
                            start=True, stop=True,
                        )
                        osb = opool.tile(
                            [128, 512], BF16, tag="osb",
                            name=f"osb{i}_{tc}_{dc2}",
                        )
                        # fp32 PSUM -> bf16 SBUF eviction, split between the
                        # two engines with headroom (Pool is too slow and
                        # back-pressures the pp ring through its queue).
                        if dc2 == 0:
                            nc.scalar.activation(
                                osb[:], po[:],
                                mybir.ActivationFunctionType.Copy,
                            )
                        else:
                            nc.vector.tensor_copy(osb[:], po[:])
                        # alternate store queues so the final drain's DMA
                        # issues overlap across two DGE pipelines
                        dq = nc.sync if dc2 == 0 else nc.gpsimd
                        dq.dma_start(
                            out[b * S + tc * 128 : b * S + (tc + 1) * 128,
                                ts(dc2, 512)],
                            osb[:],
                        )
                    return go

                for tc in range(NKC):
                    for dc2 in range(2):
                        cls.append(mk(tc, dc2))
                return cls

            def emit_window(i, b, fillers, per_step):
                """Phase i's 16 score->exp steps with fillers drained
                between steps."""
                ph = state[i]
                QT, KT = ph["QT"], ph["KT"]
                ptb = ptpool.tile(
                    [128, HPC, NKC, S], BF16, tag="pt", name=f"pt{i}"
                )
                ph["ptb"] = ptb
                late = None
                for h in range(HPC):
                    for kc in range(NKC):
                        if h == 0 and kc == 0:
                            # a few fillers ahead of the first st so the PE
                            # has work while the previous window's last exp
                            # drains the st ring slot.
                            for _ in range(per_step):
                                if fillers:
                                    fillers.popleft()()
                        st = ps_st.tile(
                            [128, S], FP32, tag="st", name=f"st{i}_{h}_{kc}"
                        )
                        for qc2 in range(NQC2):
                            nc.tensor.matmul(
                                st[:, ts(qc2, 512)],
                                lhsT=KT[ts(h, HD), ts(kc, 128)],
                                rhs=QT[ts(h, HD), ts(qc2, 512)],
                                start=True, stop=True,
                            )
                        nc.scalar.activation(ptb[:, h, kc, :], st[:], Exp)
                        if h == 1 and kc == 0 and late is None:
                            # own-phase h0 AV becomes available now
                            late = av_closures(i, 0)
                            fillers.extend(late)
                        for _ in range(per_step):
                            if fillers:
                                fillers.popleft()()
                while fillers:
                    fillers.popleft()()
                if late is None:
                    for c in av_closures(i, 0):
                        c()

            phases = [(rep, b) for rep in range(reps) for b in range(B)]
            N = len(phases)

            # prologue: phase 0 projections emitted sequentially
            for c in proj_closures(0, phases[0][1]):
                c()

            for i, (rep, b) in enumerate(phases):
                # Filler order matters for the shared pp PSUM ring:
                # proj(i+1)'s ps allocations must precede outproj(i-1)'s po
                # allocations so next-phase projections never wait on the
                # Pool-paced po eviction drain (one full window of slack).
                fillers = deque()
                if i + 1 < N:
                    fillers.extend(proj_closures(i + 1, phases[i + 1][1]))
                if i >= 1:
                    fillers.extend(av_closures(i - 1, 1))
                    fillers.extend(outproj_closures(i - 1, phases[i - 1][1]))
                per_step = max(3, (len(fillers) + 18 + 15) // 16)
                emit_window(i, b, fillers, per_step)

            # drain: last phase's second-head AV + out-projection
            for c in av_closures(N - 1, 1):
                c()
            for c in outproj_closures(N - 1, phases[N - 1][1]):
                c()

    nc.finalize()
    return nc


def _get_nc(reps=1):
    global _CACHED
    if _CACHED is None:
        _CACHED = {}
    if reps not in _CACHED:
        _CACHED[reps] = _build(reps)
    return _CACHED[reps]


def _make_in_maps(x2d, Wq, bq, Wk, bk, Wv, bv, woT_eff):
    import ml_dtypes
    bf16 = ml_dtypes.bfloat16
    sm_scale = np.float32(1.0 / np.sqrt(HD))
    xT_full = np.ascontiguousarray(x2d.T).astype(bf16)
    woT_eff = np.ascontiguousarray(woT_eff.astype(np.float32))

    def w3(WT):
        # [D, F] -> [128, 8, F]: partition p holds d-chunks {c*128+p}
        return np.ascontiguousarray(
            WT.reshape(8, 128, F).transpose(1, 0, 2)
        ).astype(bf16)

    in_maps = []
    for c in range(NCORES):
        hs = slice(c * F, (c + 1) * F)
        in_maps.append({
            "xT": xT_full,
            "wq3": w3((sm_scale * Wq[hs, :]).T),
            "wk3": w3(Wk[hs, :].T),
            "wv3": w3(Wv[hs, :].T),
            "woS": np.ascontiguousarray(woT_eff[hs, :]).astype(bf16),
            "bqs": np.ascontiguousarray((sm_scale * bq[hs])[:, None]),
            "bks": np.ascontiguousarray(bk[hs][:, None]),
            "bvb": np.ascontiguousarray(
                np.broadcast_to(bv[hs].reshape(HPC, HD)[None], (128, HPC, HD))
            ),
        })
    return in_maps


def _run_pass(x2d, Wq, bq, Wk, bk, Wv, bv, woT_eff):
    """One attention+out-projection pass on the device.

    x2d: [R, D] float32; woT_eff: [D, D] = (scale_out * Wo)^T.
    Returns [R, D] float32 = softmax(qk^T/sqrt(HD)) v @ woT_eff
    (no output bias), summed over the 8 cores' per-head partials.
    """
    global LAST
    nc = _get_nc()
    in_maps = _make_in_maps(x2d, Wq, bq, Wk, bk, Wv, bv, woT_eff)
    res = run_bass_kernel_spmd(nc, in_maps, list(range(NCORES)), trace=TRACE)
    LAST = res
    full = np.zeros((R, D), np.float32)
    for c in range(NCORES):
        full += np.asarray(res.results[c]["out"], dtype=np.float32)
    return full


def kernel(x, Wq, bq, Wk, bk, Wv, bv, Wo, bo,
           variance_reduction_weight, length_adaptive_weight, perms):
    x = np.asarray(x, dtype=np.float32)
    Wq, bq = np.asarray(Wq, np.float32), np.asarray(bq, np.float32)
    Wk, bk = np.asarray(Wk, np.float32), np.asarray(bk, np.float32)
    Wv, bv = np.asarray(Wv, np.float32), np.asarray(bv, np.float32)
    Wo, bo = np.asarray(Wo, np.float32), np.asarray(bo, np.float32)
    perms = np.asarray(perms)
    b, s, d = x.shape

    law = float(np.asarray(length_adaptive_weight).reshape(-1)[0])
    vrw = float(np.asarray(variance_reduction_weight).reshape(-1)[0])
    w = np.float32(min(max(law * np.log(s) / s, 0.01), 1.0))
    x2d = x.reshape(R, D)

    is_perm = all(
        np.array_equal(np.sort(np.asarray(perms[i])), np.arange(s))
        for i in range(perms.shape[0])
    )

    if is_perm:
        # permutation-equivariant collapse: one pass, scaled by c
        c = (1.0 - w) + w * vrw
        outp = _run_pass(x2d, Wq, bq, Wk, bk, Wv, bv, (c * Wo).T)
        outp = outp + (c * bo)[None, :]
        return outp.reshape(b, s, d).astype(np.float32)

    # general fallback: standard pass + KPERM permuted passes
    acc = _run_pass(x2d, Wq, bq, Wk, bk, Wv, bv, ((1.0 - w) * Wo).T)
    pscale = (w * vrw) / np.float32(perms.shape[0])
    for i in range(perms.shape[0]):
        perm = np.asarray(perms[i]).astype(np.int64)
        xp = x[:, perm, :].reshape(R, D)
        op = _run_pass(xp, Wq, bq, Wk, bk, Wv, bv, (pscale * Wo).T)
        op3 = op.reshape(b, s, d)
        inv = np.argsort(perm)
        acc += op3[:, inv, :].reshape(R, D)
    acc = acc + (((1.0 - w) + w * vrw) * bo)[None, :]
    return acc.reshape(b, s, d).astype(np.float32)


# revision 34
# speedup vs baseline: 1.4695x; 1.4695x over previous
# Trainium2 Bass kernel for nn_BayesianExpectationTransformerLayer.
#
# Math: attention with no positional encoding / masking is permutation-
# equivariant: _attention(x[:, perm, :]) == _attention(x)[:, perm, :].
# Hence each permuted pass, after applying the inverse permutation, equals
# the standard attention output exactly, and the whole module collapses to
#     out = c * (attention(x) @ Wo^T + bo),
#     c   = (1 - w) + w * variance_reduction_weight,
#     w   = clip(length_adaptive_weight * log(S)/S, 0.01, 1.0)
# We verify on the host that `perms` really are permutations of [0, S);
# if they are not (general fallback), we run the same device kernel once
# per pass (standard + K permuted copies) and combine on the host.
#
# Device strategy (8 NeuronCores, SPMD, tensor-parallel over heads, NO
# collective):
#   - core c owns heads 2c, 2c+1 (feature slice F = 128 of D = 1024).
#     Instead of an AllToAll re-shard for the out-projection, each core
#     computes the FULL [R, D] partial out-projection restricted to its
#     own 128 features (contraction F = 128 -> identical PE cost to a
#     1/8-rows x 1024-contraction projection) and the host sums the 8
#     bf16 partials.  No collective, no re-shard DMAs, no drain tail.
#   - per phase (= one batch): Q^T/K^T/V^T = [F, S] projections (bf16),
#     V into natural layout via PE transposes with an appended ones
#     column (softmax denominator), S^T = K Q^T scores per head, exp to
#     bf16 (no max-subtraction: |scores| < 7 for this data), AV with V
#     stationary accumulating feature-major with the denominator in row
#     HD, 1/denom applied on DVE, out-projection with at_sb token-chunks
#     stationary against the core's 128 x 1024 Wo row-slice.
#   - scheduling: the 16 st->exp score steps of phase i are ACT-bound
#     (~16 x 1.04us of exp).  All other PE work -- next phase's
#     projections+transposes, previous phase's second-head AV and
#     out-projection, this phase's first-head AV -- is emitted as
#     single-matmul "fillers" between score steps, so the PE queue never
#     drains while ACT works.  PSUM zero-region semantics allow unrelated
#     matmuls (other banks) to interleave inside accumulation chains.
#   - PSUM budget/partition: st ring 2x4KB + pp ring 2x2KB (proj ps,
#     V-transpose targets via bitcast, out-proj po) + av ring 2x2KB
#     (at_raw) = 16KB exactly.
#   - out-proj eviction fp32->bf16 on GPSIMD (Pool is otherwise idle),
#     output DMA on the DVE queue; x chunks on SP; weights on SP/ACT.

import os
import sys

for _p in ("/opt/trn_rl_repo", "/root/.axon_site/_ro/trn_rl_repo"):
    if os.path.isdir(_p) and _p not in sys.path:
        sys.path.append(_p)

from collections import deque

import numpy as np

import concourse.bass as bass
import concourse.mybir as mybir
import concourse.tile as tile
from concourse import bacc
from concourse.bass import ts
from concourse.bass_utils import run_bass_kernel_spmd
from concourse.masks import make_identity

B, S, D = 2, 1024, 1024
H, HD = 16, 64
KPERM = 20
NCORES = 8
HPC = H // NCORES          # heads per core = 2
F = HPC * HD               # per-core feature slice = 128
R = B * S                  # 2048 rows
NKC = S // 128             # 8 k-chunks per sequence
NQC2 = S // 512            # 2 q-chunks of 512
FP32 = mybir.dt.float32
BF16 = mybir.dt.bfloat16

TRACE = False              # set True to attempt an NTFF profile capture
LAST = None                # BassKernelResults of the last run

_CACHED = {}


def _build(reps=1):
    """Build the SPMD Bass program (identical on all 8 cores).

    reps > 1 repeats the whole computation serially in one NEFF (used
    only for timing: per-rep slope isolates device time from dispatch).
    """
    nc = bacc.Bacc(None)

    xT = nc.declare_dram_parameter("xT", [D, R], BF16, isOutput=False)
    # host pre-rearranges W*T [D, F] -> [128, 8, F] so each partition's
    # row is contiguous: 128 x 2KB descriptors instead of 1024 x 256B.
    wq3 = nc.declare_dram_parameter("wq3", [128, 8, F], BF16, isOutput=False)
    wk3 = nc.declare_dram_parameter("wk3", [128, 8, F], BF16, isOutput=False)
    wv3 = nc.declare_dram_parameter("wv3", [128, 8, F], BF16, isOutput=False)
    woS = nc.declare_dram_parameter("woS", [F, D], BF16, isOutput=False)
    bqs = nc.declare_dram_parameter("bqs", [F, 1], FP32, isOutput=False)
    bks = nc.declare_dram_parameter("bks", [F, 1], FP32, isOutput=False)
    bvb = nc.declare_dram_parameter("bvb", [128, HPC, HD], FP32, isOutput=False)
    out = nc.declare_dram_parameter("out", [R, D], BF16, isOutput=True)

    Exp = mybir.ActivationFunctionType.Exp

    with tile.TileContext(nc) as tc:
        with (
            # PSUM pools first so their arena layout is bank-aligned:
            # st 2x4KB | pp 2x2KB | av 2x2KB per partition.
            tc.tile_pool(name="ps_st", bufs=2, space="PSUM") as ps_st,
            tc.tile_pool(name="ps_pp", bufs=2, space="PSUM") as ps_pp,
            tc.tile_pool(name="ps_av", bufs=2, space="PSUM") as ps_av,
            tc.tile_pool(name="const", bufs=1) as cpool,
            tc.tile_pool(name="xt", bufs=10) as xtpool,
            tc.tile_pool(name="qkv", bufs=2) as qkvpool,
            tc.tile_pool(name="vnat", bufs=2) as vpool,
            tc.tile_pool(name="pt", bufs=2) as ptpool,
            tc.tile_pool(name="at", bufs=2) as atpool,
            tc.tile_pool(name="sm", bufs=4) as smpool,
            tc.tile_pool(name="aq", bufs=9) as aqpool,
            tc.tile_pool(name="osb", bufs=6) as opool,
        ):
            # ---- constants ----
            ident = cpool.tile([128, 128], FP32, tag="ident")
            make_identity(nc, ident[:])
            ident_bf = cpool.tile([128, 128], BF16, tag="ident_bf")
            nc.gpsimd.tensor_copy(ident_bf[:], ident[:])

            wq_sb = cpool.tile([128, 8, F], BF16, tag="wq")
            wk_sb = cpool.tile([128, 8, F], BF16, tag="wk")
            wv_sb = cpool.tile([128, 8, F], BF16, tag="wv")
            wo_sb = cpool.tile([128, D], BF16, tag="wo")
            # wq on SP so it lands before the first xt chunks; wk/wv/wo on
            # the ACT HWDGE queue (idle until the first exp, ~12us in) so
            # the SP queue reaches phase 0's x chunks immediately.
            nc.sync.dma_start(wq_sb[:], wq3[:])
            nc.scalar.dma_start(wk_sb[:], wk3[:])
            nc.scalar.dma_start(wv_sb[:], wv3[:])
            nc.scalar.dma_start(wo_sb[:], woS[:])
            bq_sb = cpool.tile([F, 1], FP32, tag="bq")
            bk_sb = cpool.tile([F, 1], FP32, tag="bk")
            bv_sb = cpool.tile([128, HPC, HD], FP32, tag="bv")
            nc.gpsimd.dma_start(bq_sb[:], bqs[:])
            nc.gpsimd.dma_start(bk_sb[:], bks[:])
            nc.gpsimd.dma_start(bv_sb[:], bvb[:])

            xTr = xT[:].rearrange("(c p) r -> p c r", p=128)

            state = {}

            def proj_closures(i, b):
                """Fillers that compute phase i's Q/K/V projections, V
                transposes and V-natural assembly.  One matmul each."""
                QT = qkvpool.tile([128, S], BF16, tag="QT", name=f"QT{i}")
                KT = qkvpool.tile([128, S], BF16, tag="KT", name=f"KT{i}")
                VT = qkvpool.tile([128, S], BF16, tag="VT", name=f"VT{i}")
                V0 = vpool.tile([128, NKC, HD + 1], BF16, tag="V0", name=f"V0{i}")
                V1 = vpool.tile([128, NKC, HD + 1], BF16, tag="V1", name=f"V1{i}")
                state[i] = {"QT": QT, "KT": KT, "VT": VT, "V0": V0, "V1": V1}
                cls = []

                def start_loads():
                    # [128, 2, 512] tiles: one DMA covers two d-chunks --
                    # fewer DMA issues so arrivals keep pace with the PE's
                    # 427ns-per-chunk consumption during the prologue.
                    xts = []
                    for half, rc in enumerate((2 * b, 2 * b + 1)):
                        row = []
                        for dp in range(4):
                            xt_t = xtpool.tile(
                                [128, 2, 512], BF16, tag="xt",
                                name=f"xt{i}_{half}_{dp}",
                            )
                            nc.sync.dma_start(
                                xt_t[:], xTr[:, 2 * dp : 2 * dp + 2, ts(rc, 512)]
                            )
                            row.append(xt_t)
                        xts.append(row)
                    nc.gpsimd.memset(V0[:, :, HD : HD + 1], 1.0)
                    nc.gpsimd.memset(V1[:, :, HD : HD + 1], 1.0)
                    return xts

                xts_box = {}

                def first():
                    xts_box["xts"] = start_loads()
                cls.append((0, first))

                holder = {}

                def mk_mm(half, w_sb, dc, newtile):
                    def go():
                        if newtile:
                            holder["ps"] = ps_pp.tile(
                                [128, 512], FP32, tag="pp",
                                name=f"proj{i}_{half}_{id(w_sb)}",
                            )
                        nc.tensor.matmul(
                            holder["ps"][:],
                            lhsT=w_sb[:, dc, :],
                            rhs=xts_box["xts"][half][dc // 2][:, dc % 2, :],
                            start=(dc == 0), stop=(dc == 7),
                        )
                    return go

                def mk_evict(half, b_sb, dst):
                    def go():
                        if b_sb is not None:
                            nc.vector.tensor_scalar_add(
                                dst[:, ts(half, 512)], holder["ps"][:],
                                b_sb[:, 0:1],
                            )
                        else:
                            nc.vector.tensor_copy(
                                dst[:, ts(half, 512)], holder["ps"][:]
                            )
                    return go

                for half in range(2):
                    for w_sb, b_sb, dst in (
                        (wq_sb, bq_sb, QT), (wk_sb, bk_sb, KT),
                        (wv_sb, None, VT),
                    ):
                        for dc in range(8):
                            cls.append((213, mk_mm(half, w_sb, dc, dc == 0)))
                        cls.append((0, mk_evict(half, b_sb, dst)))

                tp_box = {}

                def mk_tp(kc):
                    def go():
                        tpf = ps_pp.tile(
                            [128, 512], FP32, tag="pp", name=f"tp{i}_{kc}"
                        )
                        tpb = tpf[:].bitcast(BF16)
                        nc.tensor.transpose(
                            tpb[:, 0:128], VT[:, ts(kc, 128)], ident_bf[:]
                        )
                        tp_box[kc] = tpb
                    return go

                def mk_vadd(kc):
                    def go():
                        tpb = tp_box.pop(kc)
                        for h, Vh in ((0, V0), (1, V1)):
                            nc.vector.tensor_add(
                                Vh[:, kc, 0:HD], tpb[:, ts(h, HD)],
                                bv_sb[:, h, :],
                            )
                    return go

                for kc in range(NKC):
                    cls.append((53, mk_tp(kc)))
                    cls.append((0, mk_vadd(kc)))
                return cls

            def av_closures(i, h):
                """Fillers for phase i's AV of head h, flipped orientation:
                stationary = exp-score chunk [128 keys, 128 queries]
                (LDWEIGHTS hides fully -- measured), moving = V natural
                [keys, HD+1].  Output is query-major [128q, HD+1] with the
                softmax denominator in column HD, so 1/denom is a
                per-partition tensor_scalar multiply; a small PE transpose
                returns each chunk to feature-major at_sb for the
                out-projection.  8320+2048 PE columns vs 16384 unflipped."""
                ph = state[i]
                Vh = ph["V0"] if h == 0 else ph["V1"]
                if "at_sb" not in ph:
                    ph["at_sb"] = atpool.tile(
                        [128, S], BF16, tag="at", name=f"at{i}"
                    )
                at_sb = ph["at_sb"]
                ptb = ph["ptb"]
                cls = []
                holder = {}

                def mk_mm(qc, kc):
                    def go():
                        if kc == 0:
                            holder["aq"] = ps_av.tile(
                                [128, HD + 1], FP32, tag="av",
                                name=f"aq{i}_{h}_{qc}",
                            )
                        nc.tensor.matmul(
                            holder["aq"][:],
                            lhsT=ptb[:, h, kc, ts(qc, 128)],
                            rhs=Vh[:, kc, :],
                            start=(kc == 0), stop=(kc == 7),
                        )
                    return go

                aqs_box = {}

                def mk_norm(qc):
                    # 1/denom into query-major bf16 (DVE only, no PE)
                    def go():
                        aq = holder["aq"]
                        rcol = smpool.tile(
                            [128, 1], FP32, tag="rcol", name=f"rc{i}_{h}_{qc}"
                        )
                        nc.vector.reciprocal(rcol[:], aq[:, HD : HD + 1])
                        aqs = aqpool.tile(
                            [128, HD], BF16, tag="aq", name=f"aqs{i}_{h}_{qc}"
                        )
                        nc.vector.tensor_scalar_mul(
                            aqs[:], aq[:, 0:HD], rcol[:, 0:1]
                        )
                        aqs_box[qc] = aqs
                    return go

                def mk_tr(qc):
                    # batched at head end: by now the DVE normalize of qc is
                    # long done, so the PE never waits on the round trip
                    def go():
                        aqs = aqs_box.pop(qc)
                        tpf = ps_pp.tile(
                            [128, 512], FP32, tag="pp", name=f"atp{i}_{h}_{qc}"
                        )
                        tpb = tpf[:].bitcast(BF16)
                        nc.tensor.transpose(
                            tpb[0:HD, 0:128], aqs[:], ident_bf[:]
                        )
                        nc.vector.tensor_copy(
                            at_sb[ts(h, HD), ts(qc, 128)], tpb[0:HD, 0:128]
                        )
                    return go

                for qc in range(NKC):
                    for kc in range(NKC):
                        cls.append((27, mk_mm(qc, kc)))
                    cls.append((0, mk_norm(qc)))
                for qc in range(NKC):
                    cls.append((53, mk_tr(qc)))
                return cls

            def outproj_closures(i, b, drain=False):
                """Fillers for phase i's partial out-projection: per token
                chunk tc and 512-wide d-slice, one matmul + eviction
                + output DMA."""
                at_sb = state[i]["at_sb"]
                cls = []

                def mk(tc, dc2):
                    def go():
                        po = ps_pp.tile(
                            [128, 512], FP32, tag="pp", name=f"po{i}_{tc}_{dc2}"
                        )
                        nc.tensor.matmul(
                            po[:],
                            lhsT=at_sb[:, ts(tc, 128)],
                            rhs=wo_sb[:, ts(dc2, 512)],
                            start=True, stop=True,
                        )
                        osb = opool.tile(
                            [128, 512], BF16, tag="osb",
                            name=f"osb{i}_{tc}_{dc2}",
                        )
                        # fp32 PSUM -> bf16 SBUF eviction, on DVE in steady
                        # windows.  NOT on ACT there: the st ring couples
                        # the PE's score cadence to ACT, so any non-exp ACT
                        # work in the window stretches the whole phase.
                        # NOT on Pool: its ~0.8us ops back-pressure the pp
                        # ring.  In the final drain there are no more exps,
                        # so split ACT/DVE to halve the eviction tail.
                        if drain and dc2 == 0:
                            nc.scalar.activation(
                                osb[:], po[:],
                                mybir.ActivationFunctionType.Copy,
                            )
                        else:
                            nc.vector.tensor_copy(osb[:], po[:])
                        # alternate store queues so the final drain's DMA
                        # issues overlap across two DGE pipelines
                        dq = nc.sync if dc2 == 0 else nc.gpsimd
                        dq.dma_start(
                            out[b * S + tc * 128 : b * S + (tc + 1) * 128,
                                ts(dc2, 512)],
                            osb[:],
                        )
                    return go

                for tc in range(NKC):
                    for dc2 in range(2):
                        cls.append((213, mk(tc, dc2)))
                return cls

            def emit_window(i, b, fillers, late_cost, horder=(0, 1)):
                """Phase i's 16 score->exp steps.  Fillers are (pe_ns, fn)
                tuples; each step pops until it has banked enough PE work
                to cover the ~1.04us ACT-paced exp cadence, so the PE
                never drains while ACT works.  horder swaps head order for
                the last phase so its first-processed head's AV lands
                inside the window."""
                ph = state[i]
                QT, KT = ph["QT"], ph["KT"]
                ptb = ptpool.tile(
                    [128, HPC, NKC, S], BF16, tag="pt", name=f"pt{i}"
                )
                ph["ptb"] = ptb
                late = None

                def budget(step):
                    remaining = sum(c for c, _ in fillers)
                    if late is None:
                        remaining += late_cost
                    steps_left = max(1, 17 - step)
                    return remaining / steps_left

                def pop(target):
                    got = 0.0
                    while fillers and got < target:
                        c, fn = fillers.popleft()
                        fn()
                        got += c

                for hi_, h in enumerate(horder):
                    for kc in range(NKC):
                        step = hi_ * NKC + kc
                        if step == 0:
                            # fillers ahead of the first st so the PE has
                            # work while the previous window's last exp
                            # drains the st ring slot.
                            pop(budget(0))
                        st = ps_st.tile(
                            [128, S], FP32, tag="st", name=f"st{i}_{h}_{kc}"
                        )
                        for qc2 in range(NQC2):
                            nc.tensor.matmul(
                                st[:, ts(qc2, 512)],
                                lhsT=KT[ts(h, HD), ts(kc, 128)],
                                rhs=QT[ts(h, HD), ts(qc2, 512)],
                                start=True, stop=True,
                            )
                        nc.scalar.activation(ptb[:, h, kc, :], st[:], Exp)
                        if hi_ == 1 and kc == 0 and late is None:
                            # own-phase first-head AV becomes available now
                            late = av_closures(i, horder[0])
                            fillers.extend(late)
                        pop(budget(step + 1))
                while fillers:
                    fillers.popleft()[1]()
                if late is None:
                    for _c, fn in av_closures(i, horder[0]):
                        fn()

            phases = [(rep, b) for rep in range(reps) for b in range(B)]
            N = len(phases)

            # prologue: phase 0 projections emitted sequentially
            for _c, fn in proj_closures(0, phases[0][1]):
                fn()

            LATE_COST = 8 * (8 * 27 + 53)   # own-phase h0 AV PE ns
            for i, (rep, b) in enumerate(phases):
                # Filler order: AV(i-1,h1)'s reads of the V ring MUST be
                # emitted before proj(i+1) re-allocates those buffers (WAR);
                # po evictions are on ACT/DVE so ps(i+1) waiting on
                # po(i-1)'s ring slot drains promptly.
                fillers = deque()
                if i >= 1:
                    fillers.extend(av_closures(i - 1, 1))
                    fillers.extend(outproj_closures(i - 1, phases[i - 1][1]))
                if i + 1 < N:
                    fillers.extend(proj_closures(i + 1, phases[i + 1][1]))
                horder = (1, 0) if i == N - 1 else (0, 1)
                emit_window(i, b, fillers, LATE_COST, horder)

            # drain: last phase's remaining-head AV + out-projection with
            # ACT/DVE-split evictions (no exps left to collide with)
            for _c, fn in av_closures(N - 1, 0):
                fn()
            for _c, fn in outproj_closures(N - 1, phases[N - 1][1], drain=True):
                fn()

    nc.finalize()
    return nc


def _get_nc(reps=1):
    global _CACHED
    if _CACHED is None:
        _CACHED = {}
    if reps not in _CACHED:
        _CACHED[reps] = _build(reps)
    return _CACHED[reps]


def _make_in_maps(x2d, Wq, bq, Wk, bk, Wv, bv, woT_eff):
    import ml_dtypes
    bf16 = ml_dtypes.bfloat16
    sm_scale = np.float32(1.0 / np.sqrt(HD))
    xT_full = np.ascontiguousarray(x2d.T).astype(bf16)
    woT_eff = np.ascontiguousarray(woT_eff.astype(np.float32))

    def w3(WT):
        # [D, F] -> [128, 8, F]: partition p holds d-chunks {c*128+p}
        return np.ascontiguousarray(
            WT.reshape(8, 128, F).transpose(1, 0, 2)
        ).astype(bf16)

    in_maps = []
    for c in range(NCORES):
        hs = slice(c * F, (c + 1) * F)
        in_maps.append({
            "xT": xT_full,
            "wq3": w3((sm_scale * Wq[hs, :]).T),
            "wk3": w3(Wk[hs, :].T),
            "wv3": w3(Wv[hs, :].T),
            "woS": np.ascontiguousarray(woT_eff[hs, :]).astype(bf16),
            "bqs": np.ascontiguousarray((sm_scale * bq[hs])[:, None]),
            "bks": np.ascontiguousarray(bk[hs][:, None]),
            "bvb": np.ascontiguousarray(
                np.broadcast_to(bv[hs].reshape(HPC, HD)[None], (128, HPC, HD))
            ),
        })
    return in_maps


def _run_pass(x2d, Wq, bq, Wk, bk, Wv, bv, woT_eff):
    """One attention+out-projection pass on the device.

    x2d: [R, D] float32; woT_eff: [D, D] = (scale_out * Wo)^T.
    Returns [R, D] float32 = softmax(qk^T/sqrt(HD)) v @ woT_eff
    (no output bias), summed over the 8 cores' per-head partials.
    """
    global LAST
    nc = _get_nc()
    in_maps = _make_in_maps(x2d, Wq, bq, Wk, bk, Wv, bv, woT_eff)
    res = run_bass_kernel_spmd(nc, in_maps, list(range(NCORES)), trace=TRACE)
    LAST = res
    full = np.zeros((R, D), np.float32)
    for c in range(NCORES):
        full += np.asarray(res.results[c]["out"], dtype=np.float32)
    return full


def kernel(x, Wq, bq, Wk, bk, Wv, bv, Wo, bo,
           variance_reduction_weight, length_adaptive_weight, perms):
    x = np.asarray(x, dtype=np.float32)
    Wq, bq = np.asarray(Wq, np.float32), np.asarray(bq, np.float32)
    Wk, bk = np.asarray(Wk, np.float32), np.asarray(bk, np.float32)
    Wv, bv = np.asarray(Wv, np.float32), np.asarray(bv, np.float32)
    Wo, bo = np.asarray(Wo, np.float32), np.asarray(bo, np.float32)
    perms = np.asarray(perms)
    b, s, d = x.shape

    law = float(np.asarray(length_adaptive_weight).reshape(-1)[0])
    vrw = float(np.asarray(variance_reduction_weight).reshape(-1)[0])
    w = np.float32(min(max(law * np.log(s) / s, 0.01), 1.0))
    x2d = x.reshape(R, D)

    is_perm = all(
        np.array_equal(np.sort(np.asarray(perms[i])), np.arange(s))
        for i in range(perms.shape[0])
    )

    if is_perm:
        # permutation-equivariant collapse: one pass, scaled by c
        c = (1.0 - w) + w * vrw
        outp = _run_pass(x2d, Wq, bq, Wk, bk, Wv, bv, (c * Wo).T)
        outp = outp + (c * bo)[None, :]
        return outp.reshape(b, s, d).astype(np.float32)

    # general fallback: standard pass + KPERM permuted passes
    acc = _run_pass(x2d, Wq, bq, Wk, bk, Wv, bv, ((1.0 - w) * Wo).T)
    pscale = (w * vrw) / np.float32(perms.shape[0])
    for i in range(perms.shape[0]):
        perm = np.asarray(perms[i]).astype(np.int64)
        xp = x[:, perm, :].reshape(R, D)
        op = _run_pass(xp, Wq, bq, Wk, bk, Wv, bv, (pscale * Wo).T)
        op3 = op.reshape(b, s, d)
        inv = np.argsort(perm)
        acc += op3[:, inv, :].reshape(R, D)
    acc = acc + (((1.0 - w) + w * vrw) * bo)[None, :]
    return acc.reshape(b, s, d).astype(np.float32)
